# revision 2
# baseline (speedup 1.0000x reference)
"""Trainium2 Bass kernel for AttentionStem (sparse local 4x4-window attention).

Contract: kernel(**inputs) takes the FULL unsharded inputs (numpy, keyed as in
setup_inputs()) and returns the FULL output [4, 64, 128, 128] float32.

Algorithm (per output pixel (b, h, w), per channel o):
    q  = query_w @ x                    (1x1 conv)
    kc = key_w @ xpad                   (1x1 conv on padded grid)
    vs_k = W_k @ xpad,  W_k = sum_m softmax_m(emb)[m,k] * value_w[m]
    attn_k = softmax_k(q * kc[pix + off_k])        (16 window positions)
    out = sum_k attn_k * vs_k[pix + off_k]

Sharding: 8 cores = 4 batches x 2 H-halves (64 output rows each, 4-row halo).
Layout on chip: [128 partitions = 2 row-blocks x 64 channels, free = pixels]
with padded row stride 132 so every window shift is a contiguous slice.
Softmax is computed without max subtraction: |logit| <= |q|*|k| stays well
below exp overflow for these Gaussian-scaled inputs.
"""

import os
import sys

import numpy as np

sys.path.insert(0, "/opt/trn_rl_repo")

# Problem constants (hardcoded; kernel.py must be self-contained).
B, IC, OC, H, W = 4, 3, 64, 128, 128
KS, PAD, M = 4, 2, 4
NCORES = 8

W132 = W + 2 * PAD  # padded width = 132
SH_OUT_ROWS = 64  # output rows per core
SH_ROWS = SH_OUT_ROWS + KS  # padded input rows per core = 68
XP_FREE = SH_ROWS * W132 + 16  # xp slab free size (+pad for shifted reads)
BLK = 32  # output rows per partition-block
NBLK_FREE = BLK * W132  # 4224 free elems per block
HALF = NBLK_FREE // 2  # 2112 per half-iteration
KCV_ROWS = BLK + KS - 1  # 35 conv rows needed per block
KCV_FREE = KCV_ROWS * W132  # 4620
KCV_PAD = 16

# Config knobs (tuned after profiling).
CFG = {
    "logit_dtype": os.environ.get("K_LOGIT_DT", "f32"),  # q/kcv/L dtype
    "val_dtype": os.environ.get("K_VAL_DT", "f32"),  # vs/e/p dtype
    "acc_dtype": os.environ.get("K_ACC_DT", "f32"),  # s/num dtype
}

_CACHE = {}
LAST_RESULT = None  # BassKernelResults of the most recent run (for test.py)


def _dt(name):
    from concourse import mybir

    return {"f32": mybir.dt.float32, "bf16": mybir.dt.bfloat16}[name]


def _emit(nc, tc, xpf_ap, wts_ap, out_ap, cfg):
    """Emit the per-core program. wts: [3, 64*18] = [qw | kw | W_0..W_15]."""
    import concourse.bass as bass
    from concourse import mybir

    f32 = mybir.dt.float32
    ldt = _dt(cfg["logit_dtype"])
    vdt = _dt(cfg["val_dtype"])
    adt = _dt(cfg["acc_dtype"])
    EXP = mybir.ActivationFunctionType.Exp

    from contextlib import ExitStack

    with ExitStack() as ctx:
        const = ctx.enter_context(tc.tile_pool(name="const", bufs=1))
        qkp = ctx.enter_context(tc.tile_pool(name="qk", bufs=1))

        # ---- load inputs ----
        xpf = const.tile([IC, XP_FREE], f32)
        nc.sync.dma_start(xpf[:], xpf_ap[:])
        wts = const.tile([IC, OC * 18], f32)
        nc.sync.dma_start(wts[:], wts_ap[:])

        qw = wts[:, 0:OC]
        kw = wts[:, OC : 2 * OC]

        def conv_rows(psum_pool, dst, w_lhsT, base0, base1, total, dtype):
            """dst [128, >=total]: for blk b, partition 64b+o, free f:
            sum_c w[o,c] * xpf[c, base_b + f].  Chunked matmuls + evac."""
            off = 0
            while off < total:
                n = min(2048, total - off)
                pt = psum_pool.tile([128, 2048], f32, tag="convp")
                coff = 0
                while coff < n:
                    cn = min(512, n - coff)
                    for b, base in ((0, base0), (1, base1)):
                        nc.tensor.matmul(
                            pt[64 * b : 64 * (b + 1), coff : coff + cn],
                            w_lhsT,
                            xpf[:, base + off + coff : base + off + coff + cn],
                        )
                    coff += cn
                # evacuate PSUM -> SBUF (ScalarE: close to PSUM)
                nc.scalar.copy(dst[:, off : off + n], pt[:, :n])
                off += n

        # ---- phase 0: q and kcv convolutions ----
        q = qkp.tile([128, NBLK_FREE], ldt)
        kcv = qkp.tile([128, KCV_FREE + KCV_PAD], ldt)
        with tc.tile_pool(name="psum0", bufs=2, space="PSUM") as psum0:
            # q: output rows b*32+h, cols w -> xp free (b*32+h+2)*132 + (w+2)
            conv_rows(
                psum0, q, qw,
                (0 * BLK + PAD) * W132 + PAD,
                (1 * BLK + PAD) * W132 + PAD,
                NBLK_FREE, ldt,
            )
            # kcv: conv at padded rows [b*32, b*32+35)
            conv_rows(
                psum0, kcv, kw,
                (0 * BLK) * W132,
                (1 * BLK) * W132,
                KCV_FREE, ldt,
            )
        nc.vector.memset(kcv[:, KCV_FREE:], 0.0)

        # ---- phase 1: 16-way softmax-weighted accumulation, 2 column halves ----
        with ExitStack() as ctx1:
            vsp = ctx1.enter_context(tc.tile_pool(name="vs", bufs=3))
            psum1 = ctx1.enter_context(
                tc.tile_pool(name="psum1", bufs=2, space="PSUM")
            )
            ep = ctx1.enter_context(tc.tile_pool(name="e", bufs=3))
            tmp = ctx1.enter_context(tc.tile_pool(name="tmp", bufs=3))
            accp = ctx1.enter_context(tc.tile_pool(name="acc", bufs=1))
            outp = ctx1.enter_context(tc.tile_pool(name="out", bufs=2))

            for half in range(2):
                hoff = half * HALF
                s_acc = accp.tile([128, HALF], adt, tag="s")
                num = accp.tile([128, HALF], adt, tag="num")

                for k in range(KS * KS):
                    i, j = k // KS, k % KS
                    shift = i * W132 + j

                    # vs_k (pre-shifted): conv at xp free (b*32+i)*132 + j + f
                    vs = vsp.tile([128, HALF], vdt, tag="vs")
                    wk = wts[:, OC * (2 + k) : OC * (3 + k)]
                    off = 0
                    while off < HALF:
                        n = min(2048, HALF - off)
                        pt = psum1.tile([128, 2048], f32, tag="vsp")
                        coff = 0
                        while coff < n:
                            cn = min(512, n - coff)
                            for b in (0, 1):
                                base = (b * BLK + i) * W132 + j + hoff
                                nc.tensor.matmul(
                                    pt[64 * b : 64 * (b + 1), coff : coff + cn],
                                    wk,
                                    xpf[:, base + off + coff : base + off + coff + cn],
                                )
                            coff += cn
                        nc.scalar.copy(vs[:, off : off + n], pt[:, :n])
                        off += n

                    # L = q * kcv[shifted] ; e = exp(L)
                    L = tmp.tile([128, HALF], ldt, tag="L")
                    nc.vector.tensor_mul(
                        L[:], q[:, hoff : hoff + HALF],
                        kcv[:, shift + hoff : shift + hoff + HALF],
                    )
                    e = ep.tile([128, HALF], vdt, tag="e")
                    nc.scalar.activation(e[:], L[:], EXP)

                    # p = e * vs ; accumulate s += e, num += p
                    p = tmp.tile([128, HALF], vdt, tag="L")
                    nc.vector.tensor_mul(p[:], e[:], vs[:])
                    if k == 0:
                        nc.vector.tensor_copy(s_acc[:], e[:])
                        nc.vector.tensor_copy(num[:], p[:])
                    else:
                        nc.vector.tensor_add(s_acc[:], s_acc[:], e[:])
                        nc.vector.tensor_add(num[:], num[:], p[:])

                # out = num / s
                rinv = ep.tile([128, HALF], f32, tag="e")
                nc.vector.reciprocal(rinv[:], s_acc[:])
                o_t = outp.tile([128, HALF], f32, tag="o")
                nc.vector.tensor_mul(o_t[:], num[:], rinv[:])

                # store valid columns: half covers block rows [half*16, half*16+16)
                r0 = half * (BLK // 2)
                o_v = o_t[:].rearrange("p (h w) -> p h w", w=W132)[:, :, 0:W]
                for b in (0, 1):
                    nc.sync.dma_start(
                        out_ap[:, b * BLK + r0 : b * BLK + r0 + BLK // 2, :],
                        o_v[64 * b : 64 * (b + 1)],
                    )


def _build(cfg_key):
    if cfg_key in _CACHE:
        return _CACHE[cfg_key]
    import concourse.tile as tile
    from concourse import bacc, mybir

    nc = bacc.Bacc(
        "TRN2", target_bir_lowering=False, debug=False, num_devices=NCORES
    )
    f32 = mybir.dt.float32
    xpf_t = nc.dram_tensor("xpf", [IC, XP_FREE], f32, kind="ExternalInput")
    wts_t = nc.dram_tensor("wts", [IC, OC * 18], f32, kind="ExternalInput")
    out_t = nc.dram_tensor("out", [OC, SH_OUT_ROWS, W], f32, kind="ExternalOutput")

    with tile.TileContext(nc) as tc:
        _emit(nc, tc, xpf_t.ap(), wts_t.ap(), out_t.ap(), CFG)
    nc.compile()
    _CACHE[cfg_key] = nc
    return nc


def _host_prep(inputs):
    x = np.asarray(inputs["x"], np.float32)
    key_w = np.asarray(inputs["key_w"], np.float32)
    query_w = np.asarray(inputs["query_w"], np.float32)
    value_w = np.asarray(inputs["value_w"], np.float32)
    emb_a = np.asarray(inputs["emb_a"], np.float32)
    emb_b = np.asarray(inputs["emb_b"], np.float32)
    emb_mix = np.asarray(inputs["emb_mix"], np.float32)

    # emb softmax over m, then effective per-offset value matrices W_k [16,64,3]
    la = emb_mix @ emb_a  # (M, KS)
    lb = emb_mix @ emb_b  # (M, KS)
    eloG = (la[:, :, None] + lb[:, None, :]).reshape(M, KS * KS).astype(np.float64)
    eloG -= eloG.max(axis=0, keepdims=True)
    emb = np.exp(eloG)
    emb /= emb.sum(axis=0, keepdims=True)  # (M, 16)
    wk = np.einsum("mk,moc->koc", emb.astype(np.float32), value_w)  # (16,64,3)

    # weights tensor [3, 64*18] = [qw.T | kw.T | W_0.T .. W_15.T]
    wts = np.empty((IC, OC * 18), np.float32)
    wts[:, 0:OC] = query_w.T
    wts[:, OC : 2 * OC] = key_w.T
    for k in range(KS * KS):
        wts[:, OC * (2 + k) : OC * (3 + k)] = wk[k].T

    # padded input, shards
    xp = np.zeros((B, IC, H + 2 * PAD, W + 2 * PAD), np.float32)
    xp[:, :, PAD : PAD + H, PAD : PAD + W] = x

    in_maps = []
    for c in range(NCORES):
        b, hh = c // 2, c % 2
        slab = np.zeros((IC, XP_FREE), np.float32)
        sh = xp[b, :, hh * SH_OUT_ROWS : hh * SH_OUT_ROWS + SH_ROWS, :]
        slab[:, : SH_ROWS * W132] = sh.reshape(IC, -1)
        in_maps.append({"xpf": slab, "wts": wts})
    return in_maps


def _ensure_ntff_hook():
    """The agent image's antenv lacks axon_hooks, so boot() could not register
    the NTFF profile hook. Inject the registry module and register the
    ctypes-based hook so run_bass_kernel_spmd(trace=True) can profile."""
    import types

    try:
        import antenv
    except ImportError:
        return
    if "antenv.axon_hooks" in sys.modules:
        return
    try:
        from trn_agent_boot.trn_boot import _ntff_profile_via_ctypes

        hook = _ntff_profile_via_ctypes("/opt/axon/libaxon_pjrt.so")
    except Exception:
        hook = None
    mod = types.ModuleType("antenv.axon_hooks")
    mod._hook = hook
    mod.set_axon_ntff_profile_hook = lambda h: setattr(mod, "_hook", h)
    mod.get_axon_ntff_profile_hook = lambda: mod._hook
    sys.modules["antenv.axon_hooks"] = mod
    antenv.axon_hooks = mod


def kernel(**inputs):
    global LAST_RESULT
    in_maps = _host_prep(inputs)
    nc = _build("default")

    from concourse.bass_utils import run_bass_kernel_spmd

    trace = os.environ.get("KERNEL_TRACE", "0") == "1"
    if trace:
        _ensure_ntff_hook()
    res = run_bass_kernel_spmd(
        nc, in_maps, core_ids=list(range(NCORES)), trace=trace
    )
    LAST_RESULT = res

    out = np.empty((B, OC, H, W), np.float32)
    for c in range(NCORES):
        b, hh = c // 2, c % 2
        out[b, :, hh * SH_OUT_ROWS : (hh + 1) * SH_OUT_ROWS, :] = res.results[c]["out"]
    return out


# revision 6
# speedup vs baseline: 1.3453x; 1.3453x over previous
"""Trainium2 Bass kernel for AttentionStem (sparse local 4x4-window attention).

Contract: kernel(**inputs) takes the FULL unsharded inputs (numpy, keyed as in
setup_inputs()) and returns the FULL output [4, 64, 128, 128] float32.

Algorithm (per output pixel (b, h, w), per channel o):
    q  = query_w @ x                    (1x1 conv)
    kc = key_w @ xpad                   (1x1 conv on padded grid)
    vs_k = W_k @ xpad,  W_k = sum_m softmax_m(emb)[m,k] * value_w[m]
    attn_k = softmax_k(q * kc[pix + off_k])        (16 window positions)
    out = sum_k attn_k * vs_k[pix + off_k]

Sharding: 8 cores = 4 batches x 2 H-halves (64 output rows each, 4-row halo).
Layout on chip: [128 partitions = 2 row-blocks x 64 channels, free = pixels]
with padded row stride 132 so every window shift is a contiguous slice.
Softmax is computed without max subtraction: |logit| <= |q|*|k| stays well
below exp overflow for these Gaussian-scaled inputs.

Precision strategy: the logit convs (q, kc) run in fp32 on the TensorEngine
(exp is sensitive to absolute logit error); everything downstream runs in
bf16 on the 2x-mode VectorE path, with fp32 PSUM accumulation inside each
matmul. Measured absmax rel err vs the fp32 reference: ~1.5e-3.
"""

import os
import sys

import numpy as np

sys.path.insert(0, "/opt/trn_rl_repo")

# Problem constants (hardcoded; kernel.py must be self-contained).
B, IC, OC, H, W = 4, 3, 64, 128, 128
KS, PAD, M = 4, 2, 4
NCORES = 8

W132 = W + 2 * PAD  # padded width = 132
SH_OUT_ROWS = 64  # output rows per core
SH_ROWS = SH_OUT_ROWS + KS  # padded input rows per core = 68
XP_FREE = SH_ROWS * W132 + 16  # xp slab free size (+pad for shifted reads)
BLK = 32  # output rows per partition-block
NBLK_FREE = BLK * W132  # 4224 free elems per block
KCV_ROWS = BLK + KS - 1  # 35 conv rows needed per block
KCV_FREE = KCV_ROWS * W132  # 4620
KCV_PAD = 16

# Config knobs (tuned on hardware).
CFG = {
    "conv_qk": os.environ.get("K_CONV_QK", "f32"),  # logit conv matmul dtype
    "conv_vs": os.environ.get("K_CONV_VS", "bf16"),  # value conv matmul dtype
    "el": os.environ.get("K_EL", "bf16"),  # elementwise dtype (L/e/p/q/kcv)
    "acc": os.environ.get("K_ACC", "bf16"),  # s/num accumulator dtype
    "half": int(os.environ.get("K_HALF", "2")),  # column-split factor
}

_CACHE = {}
LAST_RESULT = None  # BassKernelResults of the most recent run (for test.py)


def _dt(name):
    from concourse import mybir

    return {
        "f32": mybir.dt.float32,
        "f32r": mybir.dt.float32r,
        "bf16": mybir.dt.bfloat16,
    }[name]


def _emit(nc, tc, aps, cfg):
    """Emit the per-core program.

    aps: dict with xpf/xpb [3, XP_FREE], wtsf/wtsb [3, 64*18], out [64,64,128].
    wts layout: [qw.T | kw.T | W_0.T .. W_15.T]."""
    from contextlib import ExitStack

    from concourse import mybir

    f32 = mybir.dt.float32
    qkdt = _dt(cfg["conv_qk"])
    vsdt = _dt(cfg["conv_vs"])
    eldt = _dt(cfg["el"])
    adt = _dt(cfg["acc"])
    EXP = mybir.ActivationFunctionType.Exp
    NH = cfg["half"]
    HF = NBLK_FREE // NH  # elementwise free size per iteration
    el_bf16 = cfg["el"] == "bf16"

    with ExitStack() as ctx:
        const = ctx.enter_context(tc.tile_pool(name="const", bufs=1))
        qkp = ctx.enter_context(tc.tile_pool(name="qk", bufs=1))

        # ---- load inputs ----
        need_f32 = "f32" in (cfg["conv_qk"], cfg["conv_vs"]) or cfg["conv_qk"] == "f32r"
        need_bf16 = "bf16" in (cfg["conv_qk"], cfg["conv_vs"])
        xp = {}
        wts = {}
        if need_f32:
            xp["f32"] = const.tile([IC, XP_FREE], f32, tag="xpf32", name="xpf32")
            nc.sync.dma_start(xp["f32"][:], aps["xpf"][:])
            wts["f32"] = const.tile([IC, OC * 18], f32, tag="wtsf32", name="wtsf32")
            nc.sync.dma_start(wts["f32"][:], aps["wtsf"][:])
            xp["f32r"] = xp["f32"].bitcast(mybir.dt.float32r)
            wts["f32r"] = wts["f32"].bitcast(mybir.dt.float32r)
        if need_bf16:
            xp["bf16"] = const.tile([IC, XP_FREE], mybir.dt.bfloat16, tag="xpbf", name="xpbf")
            nc.sync.dma_start(xp["bf16"][:], aps["xpb"][:])
            wts["bf16"] = const.tile([IC, OC * 18], mybir.dt.bfloat16, tag="wtsbf", name="wtsbf")
            nc.sync.dma_start(wts["bf16"][:], aps["wtsb"][:])

        def conv_rows(psum_pool, dst, wslot, dtname, base0, base1, total,
                      dst_off=0):
            """dst[128, f+dst_off] = sum_c w[o,c] * xp[c, base_blk + f]
            for partition 64*blk + o, f in [0, total). Chunked + evacuated."""
            xp_s = xp[dtname]
            w_l = wts[dtname][:, OC * wslot : OC * (wslot + 1)]
            off = 0
            while off < total:
                n = min(2048, total - off)
                pt = psum_pool.tile([128, 2048], f32, tag="convp")
                coff = 0
                while coff < n:
                    cn = min(512, n - coff)
                    for b, base in ((0, base0), (1, base1)):
                        nc.tensor.matmul(
                            pt[64 * b : 64 * (b + 1), coff : coff + cn],
                            w_l,
                            xp_s[:, base + off + coff : base + off + coff + cn],
                        )
                    coff += cn
                # evacuate PSUM -> SBUF (ScalarE: close to PSUM; casts dtype)
                nc.scalar.copy(
                    dst[:, dst_off + off : dst_off + off + n], pt[:, :n]
                )
                off += n

        # ---- phase 0: q and kcv convolutions (logit path) ----
        q = qkp.tile([128, NBLK_FREE], eldt, tag="q")
        # kcv with 1-element pre-pad so both parities have 4B-aligned reads:
        # kcv0 holds conv grid at [0, KCV_FREE); kcv1 = same shifted by 1.
        kcv0 = qkp.tile([128, KCV_FREE + KCV_PAD], eldt, tag="kcv0")
        if el_bf16:
            kcv1 = qkp.tile([128, KCV_FREE + KCV_PAD], eldt, tag="kcv1")
        else:
            kcv1 = None
        with tc.tile_pool(name="psum0", bufs=2, space="PSUM") as psum0:
            # q: output rows b*32+h, cols w -> xp free (b*32+h+2)*132 + (w+2)
            conv_rows(
                psum0, q, 0, cfg["conv_qk"],
                (0 * BLK + PAD) * W132 + PAD,
                (1 * BLK + PAD) * W132 + PAD,
                NBLK_FREE,
            )
            # kcv: conv at padded rows [b*32, b*32+35)
            conv_rows(
                psum0, kcv0, 1, cfg["conv_qk"],
                (0 * BLK) * W132,
                (1 * BLK) * W132,
                KCV_FREE,
            )
        nc.vector.memset(kcv0[:, KCV_FREE:], 0.0)
        if kcv1 is not None:
            # shifted copy for odd-offset reads (keeps DVE 2x mode aligned)
            nc.scalar.copy(kcv1[:, 0 : KCV_FREE + KCV_PAD - 8],
                           kcv0[:, 1 : KCV_FREE + KCV_PAD - 7])
            nc.vector.memset(kcv1[:, KCV_FREE:], 0.0)

        # ---- phase 1: 16-way softmax-weighted accumulation ----
        with ExitStack() as ctx1:
            vsp = ctx1.enter_context(tc.tile_pool(name="vs", bufs=3))
            psum1 = ctx1.enter_context(
                tc.tile_pool(name="psum1", bufs=2, space="PSUM")
            )
            ep = ctx1.enter_context(tc.tile_pool(name="e", bufs=3))
            tmp = ctx1.enter_context(tc.tile_pool(name="tmp", bufs=3))
            accp = ctx1.enter_context(tc.tile_pool(name="acc", bufs=1))
            outp = ctx1.enter_context(tc.tile_pool(name="out", bufs=2))

            for half in range(NH):
                hoff = half * HF
                s_acc = accp.tile([128, HF], adt, tag="s")
                num = accp.tile([128, HF], adt, tag="num")

                for k in range(KS * KS):
                    i, j = k // KS, k % KS
                    shift = i * W132 + j

                    # vs_k (pre-shifted): conv at xp free (b*32+i)*132+j + f
                    vs = vsp.tile([128, HF], eldt, tag="vs")
                    conv_rows(
                        psum1, vs, 2 + k, cfg["conv_vs"],
                        (0 * BLK + i) * W132 + j + hoff,
                        (1 * BLK + i) * W132 + j + hoff,
                        HF,
                    )

                    # L = q * kcv[shifted] ; e = exp(L)
                    if kcv1 is not None and (shift % 2) == 1:
                        ksrc, koff = kcv1, shift - 1
                    else:
                        ksrc, koff = kcv0, shift
                    L = tmp.tile([128, HF], eldt, tag="L")
                    nc.vector.tensor_mul(
                        L[:], q[:, hoff : hoff + HF],
                        ksrc[:, koff + hoff : koff + hoff + HF],
                    )
                    e = ep.tile([128, HF], eldt, tag="e")
                    nc.scalar.activation(e[:], L[:], EXP)

                    # p = e * vs ; accumulate s += e, num += p
                    p = tmp.tile([128, HF], eldt, tag="L")
                    nc.vector.tensor_mul(p[:], e[:], vs[:])
                    if k == 0:
                        nc.vector.tensor_copy(s_acc[:], e[:])
                        nc.vector.tensor_copy(num[:], p[:])
                    else:
                        nc.vector.tensor_add(s_acc[:], s_acc[:], e[:])
                        nc.vector.tensor_add(num[:], num[:], p[:])

                # out = num / s
                rinv = ep.tile([128, HF], f32, tag="e")
                nc.vector.reciprocal(rinv[:], s_acc[:])
                o_t = outp.tile([128, HF], f32, tag="o")
                nc.vector.tensor_mul(o_t[:], num[:], rinv[:])

                # store valid columns; this half covers block rows
                # [half*(32/NH), (half+1)*(32/NH))
                rpb = BLK // NH
                r0 = half * rpb
                o_v = o_t[:].rearrange("p (h w) -> p h w", w=W132)[:, :, 0:W]
                for b in (0, 1):
                    nc.sync.dma_start(
                        aps["out"][:, b * BLK + r0 : b * BLK + r0 + rpb, :],
                        o_v[64 * b : 64 * (b + 1)],
                    )


def _build(cfg):
    key = tuple(sorted(cfg.items()))
    if key in _CACHE:
        return _CACHE[key]
    import concourse.tile as tile
    from concourse import bacc, mybir

    nc = bacc.Bacc(
        "TRN2", target_bir_lowering=False, debug=False, num_devices=NCORES
    )
    f32 = mybir.dt.float32
    bf16 = mybir.dt.bfloat16
    aps = {}
    need_f32 = "f32" in (cfg["conv_qk"], cfg["conv_vs"]) or cfg["conv_qk"] == "f32r"
    need_bf16 = "bf16" in (cfg["conv_qk"], cfg["conv_vs"])
    if need_f32:
        aps["xpf"] = nc.dram_tensor("xpf", [IC, XP_FREE], f32,
                                    kind="ExternalInput").ap()
        aps["wtsf"] = nc.dram_tensor("wtsf", [IC, OC * 18], f32,
                                     kind="ExternalInput").ap()
    if need_bf16:
        aps["xpb"] = nc.dram_tensor("xpb", [IC, XP_FREE], bf16,
                                    kind="ExternalInput").ap()
        aps["wtsb"] = nc.dram_tensor("wtsb", [IC, OC * 18], bf16,
                                     kind="ExternalInput").ap()
    aps["out"] = nc.dram_tensor("out", [OC, SH_OUT_ROWS, W], f32,
                                kind="ExternalOutput").ap()

    with tile.TileContext(nc) as tc:
        _emit(nc, tc, aps, cfg)
    nc.compile()
    _CACHE[key] = nc
    return nc


def _host_prep(inputs, cfg):
    import ml_dtypes

    x = np.asarray(inputs["x"], np.float32)
    key_w = np.asarray(inputs["key_w"], np.float32)
    query_w = np.asarray(inputs["query_w"], np.float32)
    value_w = np.asarray(inputs["value_w"], np.float32)
    emb_a = np.asarray(inputs["emb_a"], np.float32)
    emb_b = np.asarray(inputs["emb_b"], np.float32)
    emb_mix = np.asarray(inputs["emb_mix"], np.float32)

    # emb softmax over m, then effective per-offset value matrices W_k [16,64,3]
    la = emb_mix @ emb_a  # (M, KS)
    lb = emb_mix @ emb_b  # (M, KS)
    eloG = (la[:, :, None] + lb[:, None, :]).reshape(M, KS * KS).astype(np.float64)
    eloG -= eloG.max(axis=0, keepdims=True)
    emb = np.exp(eloG)
    emb /= emb.sum(axis=0, keepdims=True)  # (M, 16)
    wk = np.einsum("mk,moc->koc", emb.astype(np.float32), value_w)  # (16,64,3)

    # weights tensor [3, 64*18] = [qw.T | kw.T | W_0.T .. W_15.T]
    wts = np.empty((IC, OC * 18), np.float32)
    wts[:, 0:OC] = query_w.T
    wts[:, OC : 2 * OC] = key_w.T
    for k in range(KS * KS):
        wts[:, OC * (2 + k) : OC * (3 + k)] = wk[k].T

    # padded input, shards
    xp = np.zeros((B, IC, H + 2 * PAD, W + 2 * PAD), np.float32)
    xp[:, :, PAD : PAD + H, PAD : PAD + W] = x

    need_f32 = "f32" in (cfg["conv_qk"], cfg["conv_vs"]) or cfg["conv_qk"] == "f32r"
    need_bf16 = "bf16" in (cfg["conv_qk"], cfg["conv_vs"])
    wtsb = wts.astype(ml_dtypes.bfloat16)

    in_maps = []
    for c in range(NCORES):
        b, hh = c // 2, c % 2
        slab = np.zeros((IC, XP_FREE), np.float32)
        sh = xp[b, :, hh * SH_OUT_ROWS : hh * SH_OUT_ROWS + SH_ROWS, :]
        slab[:, : SH_ROWS * W132] = sh.reshape(IC, -1)
        m = {}
        if need_f32:
            m["xpf"] = slab
            m["wtsf"] = wts
        if need_bf16:
            m["xpb"] = slab.astype(ml_dtypes.bfloat16)
            m["wtsb"] = wtsb
        in_maps.append(m)
    return in_maps


def _ensure_ntff_hook():
    """The agent image's antenv lacks axon_hooks, so boot() could not register
    the NTFF profile hook. Inject the registry module and register the
    ctypes-based hook so run_bass_kernel_spmd(trace=True) can profile."""
    import types

    try:
        import antenv
    except ImportError:
        return
    if "antenv.axon_hooks" in sys.modules:
        return
    try:
        from trn_agent_boot.trn_boot import _ntff_profile_via_ctypes

        hook = _ntff_profile_via_ctypes("/opt/axon/libaxon_pjrt.so")
    except Exception:
        hook = None
    mod = types.ModuleType("antenv.axon_hooks")
    mod._hook = hook
    mod.set_axon_ntff_profile_hook = lambda h: setattr(mod, "_hook", h)
    mod.get_axon_ntff_profile_hook = lambda: mod._hook
    sys.modules["antenv.axon_hooks"] = mod
    antenv.axon_hooks = mod


def kernel(**inputs):
    global LAST_RESULT
    cfg = dict(CFG)
    in_maps = _host_prep(inputs, cfg)
    nc = _build(cfg)

    from concourse.bass_utils import run_bass_kernel_spmd

    trace = os.environ.get("KERNEL_TRACE", "0") == "1"
    if trace:
        _ensure_ntff_hook()
    res = run_bass_kernel_spmd(
        nc, in_maps, core_ids=list(range(NCORES)), trace=trace
    )
    LAST_RESULT = res

    out = np.empty((B, OC, H, W), np.float32)
    for c in range(NCORES):
        b, hh = c // 2, c % 2
        out[b, :, hh * SH_OUT_ROWS : (hh + 1) * SH_OUT_ROWS, :] = res.results[c]["out"]
    return out


# revision 9
# speedup vs baseline: 1.9196x; 1.4269x over previous
"""Trainium2 Bass kernel for AttentionStem (sparse local 4x4-window attention).

Contract: kernel(**inputs) takes the FULL unsharded inputs (numpy, keyed as in
setup_inputs()) and returns the FULL output [4, 64, 128, 128] float32.

Algorithm (per output pixel (b, h, w), per channel o):
    q  = query_w @ x                    (1x1 conv)
    kc = key_w @ xpad                   (1x1 conv on padded grid)
    vs_k = W_k @ xpad,  W_k = sum_m softmax_m(emb)[m,k] * value_w[m]
    attn_k = softmax_k(q * kc[pix + off_k])        (16 window positions)
    out = sum_k attn_k * vs_k[pix + off_k]

Sharding: 8 cores = 4 batches x 2 H-halves (64 output rows each, 4-row halo).
Layout on chip: [128 partitions = 2 row-blocks x 64 channels, free = pixels]
with padded row stride 132 so every window shift is a contiguous slice.
Softmax is computed without max subtraction: |logit| <= |q|*|k| stays well
below exp overflow for these Gaussian-scaled inputs.

Precision strategy: the logit convs (q, kc) run in fp32 on the TensorEngine
(exp is sensitive to absolute logit error); everything downstream runs in
bf16 on the 2x-mode VectorE path, with fp32 PSUM accumulation inside each
matmul. Measured absmax rel err vs the fp32 reference: ~1.5e-3.
"""

import os
import sys

import numpy as np

sys.path.insert(0, "/opt/trn_rl_repo")

# Problem constants (hardcoded; kernel.py must be self-contained).
B, IC, OC, H, W = 4, 3, 64, 128, 128
KS, PAD, M = 4, 2, 4
NCORES = 8

W132 = W + 2 * PAD  # padded width = 132
SH_OUT_ROWS = 64  # output rows per core
SH_ROWS = SH_OUT_ROWS + KS  # padded input rows per core = 68
XP_FREE = SH_ROWS * W132 + 16  # xp slab free size (+pad for shifted reads)
BLK = 32  # output rows per partition-block
NBLK_FREE = BLK * W132  # 4224 free elems per block
KCV_ROWS = BLK + KS - 1  # 35 conv rows needed per block
KCV_FREE = KCV_ROWS * W132  # 4620
KCV_PAD = 16

# Config knobs (tuned on hardware).
CFG = {
    "conv_qk": os.environ.get("K_CONV_QK", "f32"),  # logit conv matmul dtype
    "conv_vs": os.environ.get("K_CONV_VS", "bf16"),  # value conv matmul dtype
    "el": os.environ.get("K_EL", "bf16"),  # elementwise dtype (L/e/p/q/kcv)
    "acc": os.environ.get("K_ACC", "bf16"),  # s/num accumulator dtype
    "half": int(os.environ.get("K_HALF", "2")),  # column-split factor
}

_CACHE = {}
LAST_RESULT = None  # BassKernelResults of the most recent run (for test.py)


def _dt(name):
    from concourse import mybir

    return {
        "f32": mybir.dt.float32,
        "f32r": mybir.dt.float32r,
        "bf16": mybir.dt.bfloat16,
    }[name]


def _emit(nc, tc, aps, cfg):
    """Emit the per-core program.

    aps: dict with xpf/xpb [3, XP_FREE], wtsf/wtsb [3, 64*18], out [64,64,128].
    wts layout: [qw.T | kw.T | W_0.T .. W_15.T]."""
    from contextlib import ExitStack

    from concourse import mybir

    f32 = mybir.dt.float32
    qkdt = _dt(cfg["conv_qk"])
    vsdt = _dt(cfg["conv_vs"])
    eldt = _dt(cfg["el"])
    adt = _dt(cfg["acc"])
    EXP = mybir.ActivationFunctionType.Exp
    NH = cfg["half"]
    HF = NBLK_FREE // NH  # elementwise free size per iteration
    el_bf16 = cfg["el"] == "bf16"

    with ExitStack() as ctx:
        const = ctx.enter_context(tc.tile_pool(name="const", bufs=1))
        qkp = ctx.enter_context(tc.tile_pool(name="qk", bufs=1))

        # ---- load inputs ----
        need_f32 = "f32" in (cfg["conv_qk"], cfg["conv_vs"]) or cfg["conv_qk"] == "f32r"
        need_bf16 = "bf16" in (cfg["conv_qk"], cfg["conv_vs"])
        xp = {}
        wts = {}
        if need_f32:
            xp["f32"] = const.tile([IC, XP_FREE], f32, tag="xpf32", name="xpf32")
            nc.sync.dma_start(xp["f32"][:], aps["xpf"][:])
            wts["f32"] = const.tile([IC, OC * 18], f32, tag="wtsf32", name="wtsf32")
            nc.sync.dma_start(wts["f32"][:], aps["wtsf"][:])
            xp["f32r"] = xp["f32"].bitcast(mybir.dt.float32r)
            wts["f32r"] = wts["f32"].bitcast(mybir.dt.float32r)
        if need_bf16:
            xp["bf16"] = const.tile([IC, XP_FREE], mybir.dt.bfloat16, tag="xpbf", name="xpbf")
            nc.sync.dma_start(xp["bf16"][:], aps["xpb"][:])
            wts["bf16"] = const.tile([IC, OC * 18], mybir.dt.bfloat16, tag="wtsbf", name="wtsbf")
            nc.sync.dma_start(wts["bf16"][:], aps["wtsb"][:])

        def conv_rows(psum_pool, dst, wslot, dtname, base0, base1, total,
                      dst_off=0, chunk=2048):
            """dst[128, f+dst_off] = sum_c w[o,c] * xp[c, base_blk + f]
            for partition 64*blk + o, f in [0, total). Chunked + evacuated."""
            xp_s = xp[dtname]
            w_l = wts[dtname][:, OC * wslot : OC * (wslot + 1)]
            off = 0
            while off < total:
                n = min(chunk, total - off)
                pt = psum_pool.tile([128, chunk], f32, tag="convp", name="cp")
                coff = 0
                while coff < n:
                    cn = min(512, n - coff)
                    for b, base in ((0, base0), (1, base1)):
                        nc.tensor.matmul(
                            pt[64 * b : 64 * (b + 1), coff : coff + cn],
                            w_l,
                            xp_s[:, base + off + coff : base + off + coff + cn],
                        )
                    coff += cn
                # evacuate PSUM -> SBUF (ScalarE: close to PSUM; casts dtype)
                nc.scalar.copy(
                    dst[:, dst_off + off : dst_off + off + n], pt[:, :n]
                )
                off += n

        # ---- phase 0: q and kcv convolutions (logit path) ----
        q = qkp.tile([128, NBLK_FREE], eldt, tag="q")
        # kcv with 1-element pre-pad so both parities have 4B-aligned reads:
        # kcv0 holds conv grid at [0, KCV_FREE); kcv1 = same shifted by 1.
        kcv0 = qkp.tile([128, KCV_FREE + KCV_PAD], eldt, tag="kcv0")
        if el_bf16:
            kcv1 = qkp.tile([128, KCV_FREE + KCV_PAD], eldt, tag="kcv1")
        else:
            kcv1 = None
        with tc.tile_pool(name="psum0", bufs=2, space="PSUM") as psum0:
            # q: output rows b*32+h, cols w -> xp free (b*32+h+2)*132 + (w+2)
            conv_rows(
                psum0, q, 0, cfg["conv_qk"],
                (0 * BLK + PAD) * W132 + PAD,
                (1 * BLK + PAD) * W132 + PAD,
                NBLK_FREE,
            )
            # kcv: conv at padded rows [b*32, b*32+35)
            conv_rows(
                psum0, kcv0, 1, cfg["conv_qk"],
                (0 * BLK) * W132,
                (1 * BLK) * W132,
                KCV_FREE,
            )
        nc.vector.memset(kcv0[:, KCV_FREE:], 0.0)
        if kcv1 is not None:
            # shifted copy for odd-offset reads (keeps DVE 2x mode aligned)
            nc.scalar.copy(kcv1[:, 0 : KCV_FREE + KCV_PAD - 8],
                           kcv0[:, 1 : KCV_FREE + KCV_PAD - 7])
            nc.vector.memset(kcv1[:, KCV_FREE:], 0.0)

        # ---- phase 1: 16-way softmax-weighted accumulation ----
        # Software-pipelined: L_{k+1} is emitted while ACT runs exp_k so the
        # VectorE never waits on the ScalarE. e_k and p_k share one [128,2*HF]
        # tile (e left, p right) so the s/num accumulation is a single add.
        with ExitStack() as ctx1:
            vsp = ctx1.enter_context(tc.tile_pool(name="vs", bufs=4))
            psum1 = ctx1.enter_context(
                tc.tile_pool(name="psum1", bufs=3, space="PSUM")
            )
            epp = ctx1.enter_context(tc.tile_pool(name="ep", bufs=3))
            tmp = ctx1.enter_context(tc.tile_pool(name="tmp", bufs=3))
            accp = ctx1.enter_context(tc.tile_pool(name="acc", bufs=1))
            finp = ctx1.enter_context(tc.tile_pool(name="fin", bufs=1))
            outp = ctx1.enter_context(tc.tile_pool(name="out", bufs=2))

            NK = KS * KS

            def vs_conv(k, half):
                i, j = k // KS, k % KS
                vs = vsp.tile([128, HF], eldt, tag="vs", name="vs")
                conv_rows(
                    psum1, vs, 2 + k, cfg["conv_vs"],
                    (0 * BLK + i) * W132 + j + half * HF,
                    (1 * BLK + i) * W132 + j + half * HF,
                    HF, chunk=1024,
                )
                return vs

            def logit_mul(k, half):
                i, j = k // KS, k % KS
                shift = i * W132 + j
                if kcv1 is not None and (shift % 2) == 1:
                    ksrc, koff = kcv1, shift - 1
                else:
                    ksrc, koff = kcv0, shift
                hoff = half * HF
                L = tmp.tile([128, HF], eldt, tag="L", name="L")
                nc.vector.tensor_mul(
                    L[:], q[:, hoff : hoff + HF],
                    ksrc[:, koff + hoff : koff + hoff + HF],
                )
                return L

            for half in range(NH):
                acc = accp.tile([128, 2 * HF], adt, tag="acc", name="acc")

                vs = vs_conv(0, half)
                L = logit_mul(0, half)
                for k in range(NK):
                    ep = epp.tile([128, 2 * HF], eldt, tag="ep", name="ep")
                    e, p = ep[:, 0:HF], ep[:, HF : 2 * HF]
                    nc.scalar.activation(e[:], L[:], EXP)
                    nc.vector.tensor_mul(p[:], e[:], vs[:])
                    if k + 1 < NK:
                        vs = vs_conv(k + 1, half)
                        L = logit_mul(k + 1, half)
                    if k == 0:
                        nc.vector.tensor_copy(acc[:], ep[:])
                    else:
                        nc.vector.tensor_add(acc[:], acc[:], ep[:])

                # out = num / s  (s needs fp32 for the bit-level recip seed)
                s_f = finp.tile([128, HF], f32, tag="sf", name="sf")
                if cfg["acc"] == "f32":
                    s_f = acc[:, 0:HF]
                else:
                    nc.scalar.copy(s_f[:], acc[:, 0:HF])
                rinv = finp.tile([128, HF], f32, tag="rinv", name="rinv")
                nc.vector.reciprocal_approx_fast(rinv[:], s_f[:])
                o_t = outp.tile([128, HF], f32, tag="o", name="o")
                nc.vector.tensor_mul(o_t[:], acc[:, HF : 2 * HF], rinv[:])

                # store valid columns; this half covers block rows
                # [half*(32/NH), (half+1)*(32/NH))
                rpb = BLK // NH
                r0 = half * rpb
                o_v = o_t[:].rearrange("p (h w) -> p h w", w=W132)[:, :, 0:W]
                for b in (0, 1):
                    nc.sync.dma_start(
                        aps["out"][:, b * BLK + r0 : b * BLK + r0 + rpb, :],
                        o_v[64 * b : 64 * (b + 1)],
                    )


def _build(cfg):
    key = tuple(sorted(cfg.items()))
    if key in _CACHE:
        return _CACHE[key]
    import concourse.tile as tile
    from concourse import bacc, mybir

    nc = bacc.Bacc(
        "TRN2", target_bir_lowering=False, debug=False, num_devices=NCORES
    )
    f32 = mybir.dt.float32
    bf16 = mybir.dt.bfloat16
    aps = {}
    need_f32 = "f32" in (cfg["conv_qk"], cfg["conv_vs"]) or cfg["conv_qk"] == "f32r"
    need_bf16 = "bf16" in (cfg["conv_qk"], cfg["conv_vs"])
    if need_f32:
        aps["xpf"] = nc.dram_tensor("xpf", [IC, XP_FREE], f32,
                                    kind="ExternalInput").ap()
        aps["wtsf"] = nc.dram_tensor("wtsf", [IC, OC * 18], f32,
                                     kind="ExternalInput").ap()
    if need_bf16:
        aps["xpb"] = nc.dram_tensor("xpb", [IC, XP_FREE], bf16,
                                    kind="ExternalInput").ap()
        aps["wtsb"] = nc.dram_tensor("wtsb", [IC, OC * 18], bf16,
                                     kind="ExternalInput").ap()
    aps["out"] = nc.dram_tensor("out", [OC, SH_OUT_ROWS, W], f32,
                                kind="ExternalOutput").ap()

    with tile.TileContext(nc) as tc:
        _emit(nc, tc, aps, cfg)
    nc.compile()
    _CACHE[key] = nc
    return nc


def _host_prep(inputs, cfg):
    import ml_dtypes

    x = np.asarray(inputs["x"], np.float32)
    key_w = np.asarray(inputs["key_w"], np.float32)
    query_w = np.asarray(inputs["query_w"], np.float32)
    value_w = np.asarray(inputs["value_w"], np.float32)
    emb_a = np.asarray(inputs["emb_a"], np.float32)
    emb_b = np.asarray(inputs["emb_b"], np.float32)
    emb_mix = np.asarray(inputs["emb_mix"], np.float32)

    # emb softmax over m, then effective per-offset value matrices W_k [16,64,3]
    la = emb_mix @ emb_a  # (M, KS)
    lb = emb_mix @ emb_b  # (M, KS)
    eloG = (la[:, :, None] + lb[:, None, :]).reshape(M, KS * KS).astype(np.float64)
    eloG -= eloG.max(axis=0, keepdims=True)
    emb = np.exp(eloG)
    emb /= emb.sum(axis=0, keepdims=True)  # (M, 16)
    wk = np.einsum("mk,moc->koc", emb.astype(np.float32), value_w)  # (16,64,3)

    # weights tensor [3, 64*18] = [qw.T | kw.T | W_0.T .. W_15.T]
    wts = np.empty((IC, OC * 18), np.float32)
    wts[:, 0:OC] = query_w.T
    wts[:, OC : 2 * OC] = key_w.T
    for k in range(KS * KS):
        wts[:, OC * (2 + k) : OC * (3 + k)] = wk[k].T

    # padded input, shards
    xp = np.zeros((B, IC, H + 2 * PAD, W + 2 * PAD), np.float32)
    xp[:, :, PAD : PAD + H, PAD : PAD + W] = x

    need_f32 = "f32" in (cfg["conv_qk"], cfg["conv_vs"]) or cfg["conv_qk"] == "f32r"
    need_bf16 = "bf16" in (cfg["conv_qk"], cfg["conv_vs"])
    wtsb = wts.astype(ml_dtypes.bfloat16)

    in_maps = []
    for c in range(NCORES):
        b, hh = c // 2, c % 2
        slab = np.zeros((IC, XP_FREE), np.float32)
        sh = xp[b, :, hh * SH_OUT_ROWS : hh * SH_OUT_ROWS + SH_ROWS, :]
        slab[:, : SH_ROWS * W132] = sh.reshape(IC, -1)
        m = {}
        if need_f32:
            m["xpf"] = slab
            m["wtsf"] = wts
        if need_bf16:
            m["xpb"] = slab.astype(ml_dtypes.bfloat16)
            m["wtsb"] = wtsb
        in_maps.append(m)
    return in_maps


def _ensure_ntff_hook():
    """The agent image's antenv lacks axon_hooks, so boot() could not register
    the NTFF profile hook. Inject the registry module and register the
    ctypes-based hook so run_bass_kernel_spmd(trace=True) can profile."""
    import types

    try:
        import antenv
    except ImportError:
        return
    if "antenv.axon_hooks" in sys.modules:
        return
    try:
        from trn_agent_boot.trn_boot import _ntff_profile_via_ctypes

        hook = _ntff_profile_via_ctypes("/opt/axon/libaxon_pjrt.so")
    except Exception:
        hook = None
    mod = types.ModuleType("antenv.axon_hooks")
    mod._hook = hook
    mod.set_axon_ntff_profile_hook = lambda h: setattr(mod, "_hook", h)
    mod.get_axon_ntff_profile_hook = lambda: mod._hook
    sys.modules["antenv.axon_hooks"] = mod
    antenv.axon_hooks = mod


def kernel(**inputs):
    global LAST_RESULT
    cfg = dict(CFG)
    in_maps = _host_prep(inputs, cfg)
    nc = _build(cfg)

    from concourse.bass_utils import run_bass_kernel_spmd

    trace = os.environ.get("KERNEL_TRACE", "0") == "1"
    if trace:
        _ensure_ntff_hook()
    res = run_bass_kernel_spmd(
        nc, in_maps, core_ids=list(range(NCORES)), trace=trace
    )
    LAST_RESULT = res

    out = np.empty((B, OC, H, W), np.float32)
    for c in range(NCORES):
        b, hh = c // 2, c % 2
        out[b, :, hh * SH_OUT_ROWS : (hh + 1) * SH_OUT_ROWS, :] = res.results[c]["out"]
    return out


# revision 13
# speedup vs baseline: 2.0470x; 1.0664x over previous
"""Trainium2 Bass kernel for AttentionStem (sparse local 4x4-window attention).

Contract: kernel(**inputs) takes the FULL unsharded inputs (numpy, keyed as in
setup_inputs()) and returns the FULL output [4, 64, 128, 128] float32.

Algorithm (per output pixel (b, h, w), per channel o):
    q  = query_w @ x                    (1x1 conv)
    kc = key_w @ xpad                   (1x1 conv on padded grid)
    vs_k = W_k @ xpad,  W_k = sum_m softmax_m(emb)[m,k] * value_w[m]
    attn_k = softmax_k(q * kc[pix + off_k])        (16 window positions)
    out = sum_k attn_k * vs_k[pix + off_k]

Sharding: 8 cores = 4 batches x 2 H-halves (64 output rows each, 4-row halo).
Layout on chip: [128 partitions = 2 row-blocks x 64 channels, free = pixels]
with padded row stride 132 so every window shift is a contiguous slice.
Softmax is computed without max subtraction: |logit| <= |q|*|k| stays well
below exp overflow for these Gaussian-scaled inputs.

Precision strategy: the logit convs (q, kc) run in fp32 on the TensorEngine
(exp is sensitive to absolute logit error); everything downstream runs in
bf16 on the 2x-mode VectorE path, with fp32 PSUM accumulation inside each
matmul. Measured absmax rel err vs the fp32 reference: ~1.5e-3.
"""

import os
import sys

import numpy as np

sys.path.insert(0, "/opt/trn_rl_repo")

# Problem constants (hardcoded; kernel.py must be self-contained).
B, IC, OC, H, W = 4, 3, 64, 128, 128
KS, PAD, M = 4, 2, 4
NCORES = 8

W132 = W + 2 * PAD  # padded width = 132
SH_OUT_ROWS = 64  # output rows per core
SH_ROWS = SH_OUT_ROWS + KS  # padded input rows per core = 68
XP_FREE = SH_ROWS * W132 + 16  # xp slab free size (+pad for shifted reads)
BLK = 32  # output rows per partition-block
NBLK_FREE = BLK * W132  # 4224 free elems per block
KCV_ROWS = BLK + KS - 1  # 35 conv rows needed per block
KCV_FREE = KCV_ROWS * W132  # 4620
KCV_PAD = 16

# Config knobs (tuned on hardware).
CFG = {
    "conv_qk": os.environ.get("K_CONV_QK", "f32"),  # logit conv matmul dtype
    "conv_vs": os.environ.get("K_CONV_VS", "bf16"),  # value conv matmul dtype
    "el": os.environ.get("K_EL", "bf16"),  # elementwise dtype (L/e/p/q/kcv)
    "acc": os.environ.get("K_ACC", "bf16"),  # s/num accumulator dtype
    "half": int(os.environ.get("K_HALF", "2")),  # column-split factor
}

_CACHE = {}
LAST_RESULT = None  # BassKernelResults of the most recent run (for test.py)


def _dt(name):
    from concourse import mybir

    return {
        "f32": mybir.dt.float32,
        "f32r": mybir.dt.float32r,
        "bf16": mybir.dt.bfloat16,
    }[name]


def _emit(nc, tc, aps, cfg):
    """Emit the per-core program.

    aps: dict with xpf/xpb [3, XP_FREE], wtsf/wtsb [3, 64*18], out [64,64,128].
    wts layout: [qw.T | kw.T | W_0.T .. W_15.T]."""
    from contextlib import ExitStack

    from concourse import mybir

    f32 = mybir.dt.float32
    qkdt = _dt(cfg["conv_qk"])
    vsdt = _dt(cfg["conv_vs"])
    eldt = _dt(cfg["el"])
    adt = _dt(cfg["acc"])
    EXP = mybir.ActivationFunctionType.Exp
    NH = cfg["half"]
    HF = NBLK_FREE // NH  # elementwise free size per iteration
    el_bf16 = cfg["el"] == "bf16"

    with ExitStack() as ctx:
        const = ctx.enter_context(tc.tile_pool(name="const", bufs=1))
        qkp = ctx.enter_context(tc.tile_pool(name="qk", bufs=1))

        # ---- load inputs ----
        need_f32 = "f32" in (cfg["conv_qk"], cfg["conv_vs"]) or cfg["conv_qk"] == "f32r"
        need_bf16 = "bf16" in (cfg["conv_qk"], cfg["conv_vs"])
        xp = {}
        wts = {}
        if need_f32:
            xp["f32"] = const.tile([IC, XP_FREE], f32, tag="xpf32", name="xpf32")
            nc.sync.dma_start(xp["f32"][:], aps["xpf"][:])
            wts["f32"] = const.tile([IC, OC * 18], f32, tag="wtsf32", name="wtsf32")
            nc.sync.dma_start(wts["f32"][:], aps["wtsf"][:])
            xp["f32r"] = xp["f32"].bitcast(mybir.dt.float32r)
            wts["f32r"] = wts["f32"].bitcast(mybir.dt.float32r)
        if need_bf16:
            xp["bf16"] = const.tile([IC, XP_FREE], mybir.dt.bfloat16, tag="xpbf", name="xpbf")
            nc.sync.dma_start(xp["bf16"][:], aps["xpb"][:])
            wts["bf16"] = const.tile([IC, OC * 18], mybir.dt.bfloat16, tag="wtsbf", name="wtsbf")
            nc.sync.dma_start(wts["bf16"][:], aps["wtsb"][:])

        def conv_rows(psum_pool, dst, wslot, dtname, base0, base1, total,
                      dst_off=0, chunk=2048):
            """dst[128, f+dst_off] = sum_c w[o,c] * xp[c, base_blk + f]
            for partition 64*blk + o, f in [0, total). Chunked + evacuated."""
            xp_s = xp[dtname]
            w_l = wts[dtname][:, OC * wslot : OC * (wslot + 1)]
            off = 0
            while off < total:
                n = min(chunk, total - off)
                pt = psum_pool.tile([128, chunk], f32, tag="convp", name="cp")
                coff = 0
                while coff < n:
                    cn = min(512, n - coff)
                    for b, base in ((0, base0), (1, base1)):
                        nc.tensor.matmul(
                            pt[64 * b : 64 * (b + 1), coff : coff + cn],
                            w_l,
                            xp_s[:, base + off + coff : base + off + coff + cn],
                        )
                    coff += cn
                # evacuate PSUM -> SBUF (ScalarE: close to PSUM; casts dtype)
                nc.scalar.copy(
                    dst[:, dst_off + off : dst_off + off + n], pt[:, :n]
                )
                off += n

        # ---- q/kcv tiles (filled per column-half so the second half's convs
        # overlap the first half's k-loop instead of serializing up front) ----
        q = qkp.tile([128, NBLK_FREE], eldt, tag="q")
        # kcv with 1-element shifted twin so both shift parities have
        # 4B-aligned reads (keeps DVE 2x mode).
        kcv0 = qkp.tile([128, KCV_FREE + KCV_PAD], eldt, tag="kcv0")
        if el_bf16:
            kcv1 = qkp.tile([128, KCV_FREE + KCV_PAD], eldt, tag="kcv1")
        else:
            kcv1 = None
        KSPLIT = 20 * W132  # kcv rows 0-19 with half 0, rows 20-34 with half 1

        def qk_phase(half, psum_pool):
            if NH != 2:
                if half > 0:
                    return
                qlo, qhi, klo, khi = 0, NBLK_FREE, 0, KCV_FREE
            elif half == 0:
                qlo, qhi, klo, khi = 0, HF, 0, KSPLIT
            else:
                qlo, qhi = half * HF, (half + 1) * HF
                klo, khi = KSPLIT, KCV_FREE
            # q: output rows b*32+h, cols w -> xp free (b*32+h+2)*132 + (w+2)
            conv_rows(
                psum_pool, q, 0, cfg["conv_qk"],
                (0 * BLK + PAD) * W132 + PAD + qlo,
                (1 * BLK + PAD) * W132 + PAD + qlo,
                qhi - qlo, dst_off=qlo, chunk=1024,
            )
            # kcv: conv at padded rows [b*32, b*32+35)
            conv_rows(
                psum_pool, kcv0, 1, cfg["conv_qk"],
                (0 * BLK) * W132 + klo,
                (1 * BLK) * W132 + klo,
                khi - klo, dst_off=klo, chunk=1024,
            )
            last = half == (1 if NH == 2 else 0)
            if last:
                nc.vector.memset(kcv0[:, KCV_FREE:], 0.0)
            if kcv1 is not None:
                if NH != 2:
                    lo, hi = 0, KCV_FREE + KCV_PAD - 8
                elif half == 0:
                    lo, hi = 0, KSPLIT - 1
                else:
                    lo, hi = KSPLIT - 1, KCV_FREE + KCV_PAD - 8
                nc.scalar.copy(kcv1[:, lo:hi], kcv0[:, lo + 1 : hi + 1])
                if last:
                    nc.vector.memset(kcv1[:, KCV_FREE:], 0.0)

        # ---- phase 1: 16-way softmax-weighted accumulation ----
        # Software-pipelined: L_{k+1} is emitted while ACT runs exp_k so the
        # VectorE never waits on the ScalarE. e_k and p_k share one [128,2*HF]
        # tile (e left, p right) so the s/num accumulation is a single add.
        with ExitStack() as ctx1:
            vsp = ctx1.enter_context(tc.tile_pool(name="vs", bufs=4))
            psum1 = ctx1.enter_context(
                tc.tile_pool(name="psum1", bufs=4, space="PSUM")
            )
            epp = ctx1.enter_context(tc.tile_pool(name="ep", bufs=3))
            tmp = ctx1.enter_context(tc.tile_pool(name="tmp", bufs=3))
            accp = ctx1.enter_context(tc.tile_pool(name="acc", bufs=1))
            finp = ctx1.enter_context(tc.tile_pool(name="fin", bufs=1))
            outp = ctx1.enter_context(tc.tile_pool(name="out", bufs=2))

            NK = KS * KS

            def vs_conv(k, half):
                i, j = k // KS, k % KS
                vs = vsp.tile([128, HF], eldt, tag="vs", name="vs")
                conv_rows(
                    psum1, vs, 2 + k, cfg["conv_vs"],
                    (0 * BLK + i) * W132 + j + half * HF,
                    (1 * BLK + i) * W132 + j + half * HF,
                    HF, chunk=1024,
                )
                return vs

            def logit_mul(k, half):
                i, j = k // KS, k % KS
                shift = i * W132 + j
                if kcv1 is not None and (shift % 2) == 1:
                    ksrc, koff = kcv1, shift - 1
                else:
                    ksrc, koff = kcv0, shift
                hoff = half * HF
                L = tmp.tile([128, HF], eldt, tag="L", name="L")
                nc.vector.tensor_mul(
                    L[:], q[:, hoff : hoff + HF],
                    ksrc[:, koff + hoff : koff + hoff + HF],
                )
                return L

            for half in range(NH):
                qk_phase(half, psum1)
                acc = accp.tile([128, 2 * HF], adt, tag="acc", name="acc")

                vs = vs_conv(0, half)
                L = logit_mul(0, half)
                for k in range(NK):
                    ep = epp.tile([128, 2 * HF], eldt, tag="ep", name="ep")
                    e, p = ep[:, 0:HF], ep[:, HF : 2 * HF]
                    nc.scalar.activation(e[:], L[:], EXP)
                    nc.vector.tensor_mul(p[:], e[:], vs[:])
                    if k + 1 < NK:
                        vs = vs_conv(k + 1, half)
                        L = logit_mul(k + 1, half)
                    if k == 0:
                        nc.vector.tensor_copy(acc[:], ep[:])
                    else:
                        nc.vector.tensor_add(acc[:], acc[:], ep[:])

                # out = num / s  (s needs fp32 for the bit-level recip seed)
                s_f = finp.tile([128, HF], f32, tag="sf", name="sf")
                if cfg["acc"] == "f32":
                    s_f = acc[:, 0:HF]
                else:
                    nc.scalar.copy(s_f[:], acc[:, 0:HF])
                rinv = finp.tile([128, HF], f32, tag="rinv", name="rinv")
                nc.vector.reciprocal_approx_fast(rinv[:], s_f[:])
                o_t = outp.tile([128, HF], f32, tag="o", name="o")
                nc.vector.tensor_mul(o_t[:], acc[:, HF : 2 * HF], rinv[:])

                # store valid columns; this half covers block rows
                # [half*(32/NH), (half+1)*(32/NH))
                rpb = BLK // NH
                r0 = half * rpb
                o_v = o_t[:].rearrange("p (h w) -> p h w", w=W132)[:, :, 0:W]
                for b in (0, 1):
                    nc.sync.dma_start(
                        aps["out"][:, b * BLK + r0 : b * BLK + r0 + rpb, :],
                        o_v[64 * b : 64 * (b + 1)],
                    )


def _build(cfg):
    key = tuple(sorted(cfg.items()))
    if key in _CACHE:
        return _CACHE[key]
    import concourse.tile as tile
    from concourse import bacc, mybir

    nc = bacc.Bacc(
        "TRN2", target_bir_lowering=False, debug=False, num_devices=NCORES
    )
    f32 = mybir.dt.float32
    bf16 = mybir.dt.bfloat16
    aps = {}
    need_f32 = "f32" in (cfg["conv_qk"], cfg["conv_vs"]) or cfg["conv_qk"] == "f32r"
    need_bf16 = "bf16" in (cfg["conv_qk"], cfg["conv_vs"])
    if need_f32:
        aps["xpf"] = nc.dram_tensor("xpf", [IC, XP_FREE], f32,
                                    kind="ExternalInput").ap()
        aps["wtsf"] = nc.dram_tensor("wtsf", [IC, OC * 18], f32,
                                     kind="ExternalInput").ap()
    if need_bf16:
        aps["xpb"] = nc.dram_tensor("xpb", [IC, XP_FREE], bf16,
                                    kind="ExternalInput").ap()
        aps["wtsb"] = nc.dram_tensor("wtsb", [IC, OC * 18], bf16,
                                     kind="ExternalInput").ap()
    aps["out"] = nc.dram_tensor("out", [OC, SH_OUT_ROWS, W], f32,
                                kind="ExternalOutput").ap()

    with tile.TileContext(nc) as tc:
        _emit(nc, tc, aps, cfg)
    nc.compile()
    _CACHE[key] = nc
    return nc


def _host_prep(inputs, cfg):
    import ml_dtypes

    x = np.asarray(inputs["x"], np.float32)
    key_w = np.asarray(inputs["key_w"], np.float32)
    query_w = np.asarray(inputs["query_w"], np.float32)
    value_w = np.asarray(inputs["value_w"], np.float32)
    emb_a = np.asarray(inputs["emb_a"], np.float32)
    emb_b = np.asarray(inputs["emb_b"], np.float32)
    emb_mix = np.asarray(inputs["emb_mix"], np.float32)

    # emb softmax over m, then effective per-offset value matrices W_k [16,64,3]
    la = emb_mix @ emb_a  # (M, KS)
    lb = emb_mix @ emb_b  # (M, KS)
    eloG = (la[:, :, None] + lb[:, None, :]).reshape(M, KS * KS).astype(np.float64)
    eloG -= eloG.max(axis=0, keepdims=True)
    emb = np.exp(eloG)
    emb /= emb.sum(axis=0, keepdims=True)  # (M, 16)
    wk = np.einsum("mk,moc->koc", emb.astype(np.float32), value_w)  # (16,64,3)

    # weights tensor [3, 64*18] = [qw.T | kw.T | W_0.T .. W_15.T]
    wts = np.empty((IC, OC * 18), np.float32)
    wts[:, 0:OC] = query_w.T
    wts[:, OC : 2 * OC] = key_w.T
    for k in range(KS * KS):
        wts[:, OC * (2 + k) : OC * (3 + k)] = wk[k].T

    # padded input, shards
    xp = np.zeros((B, IC, H + 2 * PAD, W + 2 * PAD), np.float32)
    xp[:, :, PAD : PAD + H, PAD : PAD + W] = x

    need_f32 = "f32" in (cfg["conv_qk"], cfg["conv_vs"]) or cfg["conv_qk"] == "f32r"
    need_bf16 = "bf16" in (cfg["conv_qk"], cfg["conv_vs"])
    wtsb = wts.astype(ml_dtypes.bfloat16)

    in_maps = []
    for c in range(NCORES):
        b, hh = c // 2, c % 2
        slab = np.zeros((IC, XP_FREE), np.float32)
        sh = xp[b, :, hh * SH_OUT_ROWS : hh * SH_OUT_ROWS + SH_ROWS, :]
        slab[:, : SH_ROWS * W132] = sh.reshape(IC, -1)
        m = {}
        if need_f32:
            m["xpf"] = slab
            m["wtsf"] = wts
        if need_bf16:
            m["xpb"] = slab.astype(ml_dtypes.bfloat16)
            m["wtsb"] = wtsb
        in_maps.append(m)
    return in_maps


def _ensure_ntff_hook():
    """The agent image's antenv lacks axon_hooks, so boot() could not register
    the NTFF profile hook. Inject the registry module and register the
    ctypes-based hook so run_bass_kernel_spmd(trace=True) can profile."""
    import types

    try:
        import antenv
    except ImportError:
        return
    if "antenv.axon_hooks" in sys.modules:
        return
    try:
        from trn_agent_boot.trn_boot import _ntff_profile_via_ctypes

        hook = _ntff_profile_via_ctypes("/opt/axon/libaxon_pjrt.so")
    except Exception:
        hook = None
    mod = types.ModuleType("antenv.axon_hooks")
    mod._hook = hook
    mod.set_axon_ntff_profile_hook = lambda h: setattr(mod, "_hook", h)
    mod.get_axon_ntff_profile_hook = lambda: mod._hook
    sys.modules["antenv.axon_hooks"] = mod
    antenv.axon_hooks = mod


def kernel(**inputs):
    global LAST_RESULT
    cfg = dict(CFG)
    in_maps = _host_prep(inputs, cfg)
    nc = _build(cfg)

    from concourse.bass_utils import run_bass_kernel_spmd

    trace = os.environ.get("KERNEL_TRACE", "0") == "1"
    if trace:
        _ensure_ntff_hook()
    res = run_bass_kernel_spmd(
        nc, in_maps, core_ids=list(range(NCORES)), trace=trace
    )
    LAST_RESULT = res

    out = np.empty((B, OC, H, W), np.float32)
    for c in range(NCORES):
        b, hh = c // 2, c % 2
        out[b, :, hh * SH_OUT_ROWS : (hh + 1) * SH_OUT_ROWS, :] = res.results[c]["out"]
    return out
